# revision 1
# baseline (speedup 1.0000x reference)
"""DiSAN forward kernel for 8 Trainium2 NeuronCores.

Data-parallel over batch: each of the 8 cores processes B/8 = 2 batch rows.
Inside a core, for each batch b:
  e   = gather(emb, x[b])                     # indirect DMA, [L=128, D=256]
  h   = elu(e @ Wh + Wh_b)                    # PE + ACT/DVE elu compose
  attention (both directions, per query l):
    z[d,m]  = tanh((h2[m,d] + h1[l,d] + b[d]) / c)   # ACT, bias = per-partition col
    E[d,m]  = exp(c * z)                              # ACT
    num[d]  = sum_{m in dir-slice} E * hT_masked      # DVE tensor_tensor_reduce
    den[d]  = sum_{m in dir-slice} E * keep[m]        # DVE tensor_tensor_reduce
  s = num/den with a runtime fixup: rows whose den==0 (fully masked) get the
  reference's uniform-softmax value mean_m h[m,d].
  f   = sigmoid(s @ Wf1 + h @ Wf2 + Wf2_b)    # PE, computed transposed
  u   = f*h + (1-f)*s                          # DVE, [d,l] layout
  g   = elu(u @ Ws1 + Ws1_b)                   # PE (transposed)
  as_ = g-> @ Ws + Ws_b                        # PE (transposed)
  out[b] = sum_l u * as_                       # DVE tensor_tensor_reduce

The program is mask-independent (masks enter as runtime float inputs), so one
NEFF serves all 8 cores SPMD.
"""

import functools
import numpy as np

import concourse.bass as bass
import concourse.mybir as mybir
from concourse import bacc, tile, masks
from concourse.bass_utils import run_bass_kernel_spmd

P = 128          # partitions / sequence length L
L = 128
D = 256          # model dim
D2 = 512         # 2*D
B = 16           # full batch
NCORES = 8
BLOC = B // NCORES  # batches per core
V = 32000
DT = mybir.dt.float32
F32 = mybir.dt.float32
AF = mybir.ActivationFunctionType
OP = mybir.AluOpType


def _emit_elu(nc, pool, out_ap, psum_ap, shape, tag):
    """out = elu(psum) = relu(x) + exp(min(x,0)) - 1, elementwise."""
    r = pool.tile(shape, F32, tag=tag + "_r")
    m0 = pool.tile(shape, F32, tag=tag + "_m")
    nc.scalar.activation(r[:], psum_ap, AF.Relu)
    nc.vector.tensor_scalar_min(m0[:], psum_ap, 0.0)
    nc.scalar.activation(m0[:], m0[:], AF.Exp)
    # out = (exp(min(x,0)) - 1) + relu(x)
    nc.vector.scalar_tensor_tensor(out_ap, m0[:], 1.0, r[:], OP.subtract, OP.add)


def build_nc(c_val: float, reps: int = 1):
    STAGE = 99
    SKIP = set()
    LLIM = L
    nc = bacc.Bacc("TRN2", target_bir_lowering=False)

    x_d = nc.dram_tensor("x_idx", [BLOC, P], mybir.dt.int32, kind="ExternalInput")
    emb_d = nc.dram_tensor("emb", [V, D], F32, kind="ExternalInput")
    whw_d = nc.dram_tensor("wh_w", [D, D], F32, kind="ExternalInput")
    whb_d = nc.dram_tensor("wh_b", [1, D], F32, kind="ExternalInput")
    w1w_d = nc.dram_tensor("w1_w", [D, D], F32, kind="ExternalInput")
    w2w_d = nc.dram_tensor("w2_w", [D, D], F32, kind="ExternalInput")
    batt_d = nc.dram_tensor("b_att", [1, D], F32, kind="ExternalInput")
    wf1_d = nc.dram_tensor("wf1_w", [D, D], F32, kind="ExternalInput")
    wf2_d = nc.dram_tensor("wf2_w", [D, D], F32, kind="ExternalInput")
    wf2b_d = nc.dram_tensor("wf2_b", [1, D], F32, kind="ExternalInput")
    ws1_d = nc.dram_tensor("ws1_w", [D2, D2], F32, kind="ExternalInput")
    ws1b_d = nc.dram_tensor("ws1_b", [1, D2], F32, kind="ExternalInput")
    ws_d = nc.dram_tensor("ws_w", [D2, D2], F32, kind="ExternalInput")
    wsb_d = nc.dram_tensor("ws_b", [1, D2], F32, kind="ExternalInput")
    kv_d = nc.dram_tensor("kv", [BLOC, P], F32, kind="ExternalInput")  # 1=keep 0=pad
    out_d = nc.dram_tensor("out", [BLOC, D2], F32, kind="ExternalOutput")

    ic = 1.0 / c_val

    with tile.TileContext(nc) as tc:
        with (
            tc.tile_pool(name="wpool", bufs=1) as wp,
            tc.tile_pool(name="bpool", bufs=2) as bp,
            tc.tile_pool(name="epool", bufs=2) as ep,
            tc.tile_pool(name="scratch", bufs=12) as sp,
            tc.tile_pool(name="psum", bufs=3, space="PSUM") as pp,
        ):
            # ---- constants / weights in SBUF ----
            ident = wp.tile([P, P], F32)
            masks.make_identity(nc, ident[:])
            ones1 = wp.tile([1, P], F32)
            nc.gpsimd.memset(ones1[:], 1.0)

            def load_w(dram, kc, n):  # [kc*128, n] -> sbuf [128, kc, n]
                t = wp.tile([P, kc, n], F32, tag="w_" + dram.name)
                nc.sync.dma_start(t[:], dram.rearrange("(c p) n -> p c n", p=P))
                return t

            whw = load_w(whw_d, 2, D)
            w1w = load_w(w1w_d, 2, D)
            w2w = load_w(w2w_d, 2, D)
            wf1 = load_w(wf1_d, 2, D)
            wf2 = load_w(wf2_d, 2, D)
            ws1 = load_w(ws1_d, 4, D2)
            wsw = load_w(ws_d, 4, D2)

            def load_row(dram, n):  # [1, n] row in sbuf
                t = wp.tile([1, n], F32, tag="r_" + dram.name)
                nc.sync.dma_start(t[:], dram[:])
                return t

            whb = load_row(whb_d, D)
            wf2b = load_row(wf2b_d, D)
            ws1b = load_row(ws1b_d, D2)
            wsb = load_row(wsb_d, D2)

            # b_att as per-partition columns, pre-scaled by 1/c: [128, 2]
            bc_col = wp.tile([P, 2], F32)
            nc.sync.dma_start(bc_col[:], batt_d.rearrange("o (c p) -> p (o c)", p=P))
            nc.vector.tensor_scalar_mul(bc_col[:], bc_col[:], ic)

            for rep in range(reps):
              for bi in range(BLOC):
                # ---- embedding gather ----
                xidx = bp.tile([P, 1], mybir.dt.int32, tag="xidx")
                nc.sync.dma_start(xidx[:], x_d[bi : bi + 1, :].rearrange("o p -> p o"))
                e_sb = bp.tile([P, D], F32, tag="e_sb")
                nc.gpsimd.indirect_dma_start(
                    out=e_sb[:],
                    out_offset=None,
                    in_=emb_d[:],
                    in_offset=bass.IndirectOffsetOnAxis(ap=xidx[:, :1], axis=0),
                )

                kvcol = bp.tile([P, 1], F32, tag="kvcol")
                nc.sync.dma_start(kvcol[:], kv_d[bi : bi + 1, :].rearrange("o p -> p o"))
                kvrow = bp.tile([1, P], F32, tag="kvrow")
                nc.sync.dma_start(kvrow[:], kv_d[bi : bi + 1, :])

                # ---- eT ----
                eT = bp.tile([P, 2, P], F32, tag="eT")
                for hf in range(2):
                    pt = pp.tile([P, P], F32, tag="t128")
                    nc.tensor.matmul(pt[:], e_sb[:, hf * P : (hf + 1) * P], ident[:], is_transpose=True)
                    nc.scalar.activation(eT[:, hf, :], pt[:], AF.Copy)

                # ---- h = elu(e @ Wh + whb) ----
                ph = pp.tile([P, D], F32, tag="t256")
                nc.tensor.matmul(ph[:], eT[:, 0, :], whw[:, 0, :], start=True, stop=False)
                nc.tensor.matmul(ph[:], eT[:, 1, :], whw[:, 1, :], start=False, stop=False)
                nc.tensor.matmul(ph[:], ones1[:], whb[:], start=False, stop=True)
                h = bp.tile([P, D], F32, tag="h")
                _emit_elu(nc, sp, h[:], ph[:], [P, D], "eluh")
                if STAGE < 2:
                    nc.sync.dma_start(out_d[bi : bi + 1, :].rearrange("o (c p) -> p (o c)", p=P), h[:, :4])
                    continue

                # ---- hT, masked h, hTm, sumh ----
                hT = bp.tile([P, 2, P], F32, tag="hT")
                for hf in range(2):
                    pt = pp.tile([P, P], F32, tag="t128")
                    nc.tensor.matmul(pt[:], h[:, hf * P : (hf + 1) * P], ident[:], is_transpose=True)
                    nc.scalar.activation(hT[:, hf, :], pt[:], AF.Copy)
                if "hm" not in SKIP:
                    hTm = bp.tile([P, 2, P], F32, tag="hTm")
                    hm = sp.tile([P, D], F32, tag="hm")
                    nc.vector.tensor_scalar_mul(hm[:], h[:], kvcol[:, :1])
                    for hf in range(2):
                        pt = pp.tile([P, P], F32, tag="t128")
                        nc.tensor.matmul(pt[:], hm[:, hf * P : (hf + 1) * P], ident[:], is_transpose=True)
                        nc.scalar.activation(hTm[:, hf, :], pt[:], AF.Copy)
                if "sumh" not in SKIP:
                    sumh = bp.tile([P, 2], F32, tag="sumh")
                    for hf in range(2):
                        nc.vector.tensor_reduce(sumh[:, hf : hf + 1], hT[:, hf, :], mybir.AxisListType.X, OP.add)

                # ---- h2Tc = (W2.T h.T)/c ; h1bT = (W1.T h.T + b)/c ----
                if "h2" not in SKIP:
                    h2Tc = bp.tile([P, 2, P], F32, tag="h2Tc")
                    h1bT = bp.tile([P, 2, P], F32, tag="h1bT")
                for hf in range(2 if "h2" not in SKIP else 0):
                    p2 = pp.tile([P, P], F32, tag="t128")
                    nc.tensor.matmul(p2[:], w2w[:, 0, hf * P : (hf + 1) * P], hT[:, 0, :], start=True, stop=False)
                    nc.tensor.matmul(p2[:], w2w[:, 1, hf * P : (hf + 1) * P], hT[:, 1, :], start=False, stop=True)
                    nc.scalar.activation(h2Tc[:, hf, :], p2[:], AF.Copy, scale=ic)
                    p1 = pp.tile([P, P], F32, tag="t128")
                    nc.tensor.matmul(p1[:], w1w[:, 0, hf * P : (hf + 1) * P], hT[:, 0, :], start=True, stop=False)
                    nc.tensor.matmul(p1[:], w1w[:, 1, hf * P : (hf + 1) * P], hT[:, 1, :], start=False, stop=True)
                    nc.scalar.activation(
                        h1bT[:, hf, :], p1[:], AF.Identity, scale=ic, bias=bc_col[:, hf : hf + 1]
                    )

                # ---- QK: key-keep broadcast [d, m] = kv[m] ----
                if "qk" not in SKIP:
                    QK = bp.tile([P, P], F32, tag="QK")
                    pqk = pp.tile([P, P], F32, tag="t128")
                    nc.tensor.matmul(pqk[:], ones1[:], kvrow[:], start=True, stop=True)
                    nc.scalar.activation(QK[:], pqk[:], AF.Copy)

                # ---- attention accumulators ----
                if STAGE < 3:
                    nc.sync.dma_start(out_d[bi : bi + 1, :].rearrange("o (c p) -> p (o c)", p=P), hT[:, 0, :4])
                    continue
                SN = bp.tile([P, 2, 2, P], F32, tag="SN")  # [d, half, dir, l]
                SD = bp.tile([P, 2, 2, P], F32, tag="SD")
                nc.gpsimd.memset(SN[:], 0.0)
                nc.gpsimd.memset(SD[:], 0.0)

                # ---- attention inner loop (tanh/exp grouped over G queries) ----
                G = 16
                for g0 in range(0, LLIM, G):
                    ng = min(G, LLIM - g0)
                    et = ep.tile([P, 2, G, P], F32, tag="et")
                    for hf in range(2):
                        zin = ep.tile([P, G, P], F32, tag="zin")
                        nc.gpsimd.tensor_tensor(
                            zin[:, :ng, :],
                            h2Tc[:, hf, :].unsqueeze(1).to_broadcast([P, ng, P]),
                            h1bT[:, hf, g0 : g0 + ng].unsqueeze(2).to_broadcast([P, ng, P]),
                            OP.add,
                        )
                        nc.scalar.activation(zin[:, :ng, :], zin[:, :ng, :], AF.Tanh)
                        nc.scalar.activation(et[:, hf, :ng, :], zin[:, :ng, :], AF.Exp, scale=c_val)
                    for gi in range(ng):
                        li = g0 + gi
                        for hf in range(2):
                            for dr, sl in ((0, slice(li + 1, L)), (1, slice(0, li))):
                                w = sl.stop - sl.start
                                if w <= 0:
                                    continue
                                scr = sp.tile([P, P], F32, tag="ttr")
                                nc.vector.scalar_tensor_tensor(
                                    scr[:, :w], et[:, hf, gi, sl], 1.0, hTm[:, hf, sl],
                                    OP.mult, OP.mult,
                                    accum_out=SN[:, hf, dr, li : li + 1],
                                )
                                scr2 = sp.tile([P, P], F32, tag="ttr")
                                nc.vector.scalar_tensor_tensor(
                                    scr2[:, :w], et[:, hf, gi, sl], 1.0, QK[:, sl],
                                    OP.mult, OP.mult,
                                    accum_out=SD[:, hf, dr, li : li + 1],
                                )

                if STAGE < 4:
                    nc.sync.dma_start(out_d[bi : bi + 1, :].rearrange("o (c p) -> p (o c)", p=P), SN[:, 0, 0, :4])
                    continue
                # ---- s = num/den with den==0 -> uniform (mean_m h) fixup ----
                S = bp.tile([P, 2, 2, P], F32, tag="S")
                for hf in range(2):
                    sn2 = SN[:, hf, :, :]  # [128, 2, 128]
                    sd2 = SD[:, hf, :, :]
                    for dr in range(2):
                        nc.vector.tensor_mul(SN[:, hf, dr, :], SN[:, hf, dr, :], QK[:])
                        nc.vector.tensor_mul(SD[:, hf, dr, :], SD[:, hf, dr, :], QK[:])
                    flag = sp.tile([P, 2, P], F32, tag="flag")
                    nc.vector.tensor_scalar(flag[:], sd2, 0.0, None, OP.is_equal)
                    nc.vector.scalar_tensor_tensor(sd2, flag[:], float(L), sd2, OP.mult, OP.add)
                    tmp = sp.tile([P, 2, P], F32, tag="flag")
                    nc.vector.tensor_scalar_mul(tmp[:], flag[:], sumh[:, hf : hf + 1])
                    nc.vector.tensor_add(sn2, sn2, tmp[:])
                    rd = sp.tile([P, 2, P], F32, tag="flag")
                    nc.vector.reciprocal(rd[:], sd2)
                    nc.vector.tensor_tensor(S[:, hf, :, :], sn2, rd[:], OP.mult)

                if STAGE < 5:
                    nc.sync.dma_start(out_d[bi : bi + 1, :].rearrange("o (c p) -> p (o c)", p=P), S[:, 0, 0, :4])
                    continue
                # ---- fT, uT per direction ----
                UT = bp.tile([P, 4, P], F32, tag="UT")  # k-chunks: fw0 fw1 bw0 bw1
                for dr in range(2):
                    fT = sp.tile([P, 2, P], F32, tag="fT")
                    for hf in range(2):
                        pf = pp.tile([P, P], F32, tag="t128")
                        nc.tensor.matmul(pf[:], wf1[:, 0, hf * P : (hf + 1) * P], S[:, 0, dr, :], start=True, stop=False)
                        nc.tensor.matmul(pf[:], wf1[:, 1, hf * P : (hf + 1) * P], S[:, 1, dr, :], start=False, stop=False)
                        nc.tensor.matmul(pf[:], wf2[:, 0, hf * P : (hf + 1) * P], hT[:, 0, :], start=False, stop=False)
                        nc.tensor.matmul(pf[:], wf2[:, 1, hf * P : (hf + 1) * P], hT[:, 1, :], start=False, stop=False)
                        nc.tensor.matmul(pf[:], wf2b[:, hf * P : (hf + 1) * P], ones1[:], start=False, stop=True)
                        nc.scalar.activation(fT[:, hf, :], pf[:], AF.Sigmoid)
                    for hf in range(2):
                        d1 = sp.tile([P, P], F32, tag="u1")
                        nc.vector.tensor_sub(d1[:], hT[:, hf, :], S[:, hf, dr, :])
                        nc.vector.tensor_mul(d1[:], fT[:, hf, :], d1[:])
                        nc.vector.tensor_add(UT[:, dr * 2 + hf, :], d1[:], S[:, hf, dr, :])

                if STAGE < 6:
                    nc.sync.dma_start(out_d[bi : bi + 1, :].rearrange("o (c p) -> p (o c)", p=P), UT[:, 0, :4])
                    continue
                # ---- gT = elu(Ws1.T u.T + ws1b) ----
                gT = bp.tile([P, 4, P], F32, tag="gT")
                for jc in range(4):
                    pg = pp.tile([P, P], F32, tag="t128")
                    for kc in range(4):
                        nc.tensor.matmul(
                            pg[:], ws1[:, kc, jc * P : (jc + 1) * P], UT[:, kc, :],
                            start=(kc == 0), stop=False,
                        )
                    nc.tensor.matmul(pg[:], ws1b[:, jc * P : (jc + 1) * P], ones1[:], start=False, stop=True)
                    _emit_elu(nc, sp, gT[:, jc, :], pg[:], [P, P], "elug")

                # ---- att_sT and final reduction ----
                outc = bp.tile([P, 4], F32, tag="outc")
                for jc in range(4):
                    pa = pp.tile([P, P], F32, tag="t128")
                    for kc in range(4):
                        nc.tensor.matmul(
                            pa[:], wsw[:, kc, jc * P : (jc + 1) * P], gT[:, kc, :],
                            start=(kc == 0), stop=False,
                        )
                    nc.tensor.matmul(pa[:], wsb[:, jc * P : (jc + 1) * P], ones1[:], start=False, stop=True)
                    scr = sp.tile([P, P], F32, tag="fin")
                    nc.vector.scalar_tensor_tensor(
                        scr[:], UT[:, jc, :], 1.0, pa[:],
                        OP.mult, OP.mult,
                        accum_out=outc[:, jc : jc + 1],
                    )

                nc.sync.dma_start(
                    out_d[bi : bi + 1, :].rearrange("o (c p) -> p (o c)", p=P), outc[:]
                )

    nc.compile()
    return nc


@functools.lru_cache(maxsize=6)
def _cached_nc(c_val: float, reps: int = 1):
    return build_nc(c_val, reps)


def build_in_maps(inputs):
    x = np.asarray(inputs["x"])
    mask = np.asarray(inputs["mask"])
    f32 = lambda a: np.ascontiguousarray(np.asarray(a), dtype=np.float32)
    common = {
        "emb": f32(inputs["emb"]),
        "wh_w": f32(inputs["Wh_w"]), "wh_b": f32(inputs["Wh_b"]).reshape(1, D),
        "w1_w": f32(inputs["W1_w"]), "w2_w": f32(inputs["W2_w"]),
        "b_att": f32(inputs["b"]).reshape(1, D),
        "wf1_w": f32(inputs["Wf1_w"]), "wf2_w": f32(inputs["Wf2_w"]),
        "wf2_b": f32(inputs["Wf2_b"]).reshape(1, D),
        "ws1_w": f32(inputs["Ws1_w"]), "ws1_b": f32(inputs["Ws1_b"]).reshape(1, D2),
        "ws_w": f32(inputs["Ws_w"]), "ws_b": f32(inputs["Ws_b"]).reshape(1, D2),
    }
    kv_full = (~mask).astype(np.float32)  # 1.0 = keep, 0.0 = pad
    in_maps = []
    for ci in range(NCORES):
        sl = slice(ci * BLOC, (ci + 1) * BLOC)
        in_maps.append({
            **common,
            "x_idx": np.ascontiguousarray(x[sl].astype(np.int32)),
            "kv": np.ascontiguousarray(kv_full[sl]),
        })
    return in_maps


def kernel(x, mask, emb, Wh_w, Wh_b, W1_w, W2_w, b, c, Wf1_w, Wf2_w, Wf2_b,
           Ws1_w, Ws1_b, Ws_w, Ws_b):
    c_val = float(np.asarray(c).reshape(-1)[0])
    nc = _cached_nc(c_val)
    in_maps = build_in_maps({
        "x": x, "mask": mask, "emb": emb, "Wh_w": Wh_w, "Wh_b": Wh_b,
        "W1_w": W1_w, "W2_w": W2_w, "b": b, "Wf1_w": Wf1_w, "Wf2_w": Wf2_w,
        "Wf2_b": Wf2_b, "Ws1_w": Ws1_w, "Ws1_b": Ws1_b, "Ws_w": Ws_w, "Ws_b": Ws_b,
    })
    res = run_bass_kernel_spmd(nc, in_maps, list(range(NCORES)))
    globals()["last_results"] = res
    out = np.concatenate([res.results[i]["out"] for i in range(NCORES)], axis=0)
    return out.astype(np.float32)

